# revision 13
# baseline (speedup 1.0000x reference)
"""Gaunt tensor product kernel for Trainium2 (8 NeuronCores, SPMD over N).

Math: the reference's synthesize->pointwise-multiply->analyze pipeline over a
(20,19) Gauss-Legendre x uniform grid is an exact quadrature for bandlimited
signals, so it collapses to the sparse Gaunt contraction
    z[n,c,d] = sum_{e,f} G[d,e,f] * xc[n,c,e] * yc[n,c,f]
with G the real-SH Gaunt coefficients (162 nonzeros of 25*81).

Per-core dataflow (128 nodes/core), single SBUF layout with partitions =
mul/C and free = (block, n):
  1. DMA x,y contiguously as [n, (m,e)], PE-transpose per e -> [m, (e,n)]
  2. S1: per-l matmuls lhsT=wx[l] -> xc [c, (e,n)]   (scale folded into wx)
  3. Gaunt middle on DVE: 81 products + 162 scaled accumulates -> z [c,(d,n)]
  4. S4: per-l matmuls lhsT=wz[l] -> zz [c', (d,n)]
  5. PE-transpose per d -> out_s [n, (d,c)], single contiguous-DRAM DMA out.
"""

import math

import numpy as np

N, MUL, C = 1024, 128, 128
L1, L2, LZ = 2, 2, 4
RB, RA = 20, 19
NCORES = 8
NPC = N // NCORES  # nodes per core
DX = (L1 + 1) ** 2  # 9
DZ = (LZ + 1) ** 2  # 25


# ---------------------------------------------------------------- Gaunt table
def _assoc_legendre(l, m, x):
    pmm = np.ones_like(x)
    if m > 0:
        dfact = float(np.prod(np.arange(1, 2 * m, 2)))
        pmm = ((-1.0) ** m) * dfact * (1.0 - x * x) ** (m / 2.0)
    if l == m:
        return pmm
    pmmp1 = x * (2 * m + 1) * pmm
    if l == m + 1:
        return pmmp1
    for ll in range(m + 2, l + 1):
        p = ((2 * ll - 1) * x * pmmp1 - (ll + m - 1) * pmm) / (ll - m)
        pmm, pmmp1 = pmmp1, p
    return pmmp1


def _sh_table(L, rb, ra):
    ct, qw = np.polynomial.legendre.leggauss(rb)
    alpha = 2.0 * np.pi * np.arange(ra) / ra
    Y = np.zeros(((L + 1) ** 2, rb, ra))
    for l in range(L + 1):
        for m in range(-l, l + 1):
            am = abs(m)
            nlm = math.sqrt((2 * l + 1) / (4 * math.pi)
                            * math.factorial(l - am) / math.factorial(l + am))
            P = _assoc_legendre(l, am, ct)
            if m == 0:
                ang = np.ones(ra)
            elif m > 0:
                ang = math.sqrt(2.0) * np.cos(m * alpha)
            else:
                ang = math.sqrt(2.0) * np.sin(am * alpha)
            Y[l * l + l + m] = nlm * P[:, None] * ang[None, :]
    return Y, qw


def _gaunt_entries():
    """Returns list of (d, e, f, g) with g = nonzero Gaunt coefficient."""
    Y, qw = _sh_table(LZ, RB, RA)
    G = np.einsum('dba,eba,fba,b->def', Y, Y[:DX], Y[:DX], qw) * (2.0 * np.pi / RA)
    G[np.abs(G) < 1e-10] = 0.0
    entries = []
    for d in range(DZ):
        for e in range(DX):
            for f in range(DX):
                if G[d, e, f] != 0.0:
                    entries.append((d, e, f, float(G[d, e, f])))
    return entries


_ENTRIES = _gaunt_entries()

_NC_CACHE = None


# ---------------------------------------------------------------- Bass kernel
def _build():
    global _NC_CACHE
    if _NC_CACHE is not None:
        return _NC_CACHE
    import concourse.bass as bass
    import concourse.mybir as mybir
    from concourse.tile import TileContext

    f32 = mybir.dt.float32
    nc = bass.Bass()

    # single input blob: per-partition [x (m,e) 1152 | y 1152 | w 12*128]
    # where w packs [wx(3) | wy(3) | wz(5) | identity(1)] as [128,128] blocks
    FB = MUL * DX * 2 + 12 * 128
    in_d = nc.declare_dram_parameter("inp", [128, FB], f32, isOutput=False)
    out_d = nc.declare_dram_parameter("out", [NPC, C, DZ], f32, isOutput=True)

    # l-block column offsets within the 9/25-wide coefficient axes
    xlb = [(l * l, (l + 1) ** 2) for l in range(L1 + 1)]   # [(0,1),(1,4),(4,9)]
    zlb = [(l * l, (l + 1) ** 2) for l in range(LZ + 1)]

    with TileContext(nc) as tc:
        with (
            tc.tile_pool(name="consts", bufs=1) as consts,
            tc.tile_pool(name="big", bufs=1) as big,
            tc.tile_pool(name="pst", bufs=2, space="PSUM") as pst,
            tc.tile_pool(name="psm", bufs=4, space="PSUM") as psm,
        ):
            blob = big.tile([128, FB], f32, tag="blob")
            nc.sync.dma_start(out=blob[:], in_=in_d[:, :])
            x_s = blob[:, 0:MUL * DX]
            y_s = blob[:, MUL * DX:2 * MUL * DX]
            w0 = 2 * MUL * DX
            wx_s = blob[:, w0:w0 + 3 * C]
            wy_s = blob[:, w0 + 3 * C:w0 + 6 * C]
            wz_s = blob[:, w0 + 6 * C:w0 + 11 * C]
            ident = blob[:, w0 + 11 * C:w0 + 12 * C]

            # Matmult instructions support a single sync-wait; pre-touch the
            # DMA'd blob from PE via a dummy transpose so real matmuls only
            # ever wait on one compute engine.
            sc = pst.tile([128, 128], f32, tag="pst")
            nc.tensor.transpose(sc[:], blob[:, :128], ident)

            # ---- stage A: transpose [n,(m,e)] -> [m,(e,n)] (9 per input)
            xt_s = big.tile([128, DX * NPC], f32, tag="xt_s")
            yt_s = big.tile([128, DX * NPC], f32, tag="yt_s")
            for src, dst in ((x_s, xt_s), (y_s, yt_s)):
                src3 = src.rearrange("n (m e) -> n m e", e=DX)
                for e in range(DX):
                    ps = pst.tile([128, 128], f32, tag="pst")
                    nc.tensor.transpose(ps[:], src3[:, :, e], ident)
                    nc.vector.tensor_copy(out=dst[:, e * NPC:(e + 1) * NPC], in_=ps[:])

            # ---- stage B: S1 per-l matmuls -> xc/yc [c, (e,n)]
            xc_s = big.tile([128, DX * NPC], f32, tag="xc_s")
            yc_s = big.tile([128, DX * NPC], f32, tag="yc_s")
            for wxy, t_s, c_s in ((wx_s, xt_s, xc_s), (wy_s, yt_s, yc_s)):
                for l, (b0, b1) in enumerate(xlb):
                    w = (b1 - b0) * NPC
                    off = b0 * NPC
                    for ch0 in range(0, w, 512):
                        cw = min(512, w - ch0)
                        ps = psm.tile([128, 512], f32, tag="psm")
                        nc.tensor.matmul(
                            ps[:, :cw],
                            wxy[:, l * C:(l + 1) * C],
                            t_s[:, off + ch0: off + ch0 + cw],
                            start=True, stop=True,
                        )
                        nc.vector.tensor_copy(
                            out=c_s[:, off + ch0: off + ch0 + cw], in_=ps[:, :cw]
                        )

            # ---- stage C: Gaunt middle on DVE
            q_s = big.tile([128, DX * DX * NPC], f32, tag="q_s")
            z_s = big.tile([128, DZ * NPC], f32, tag="z_s")
            for e in range(DX):
                for f in range(DX):
                    nc.vector.tensor_mul(
                        out=q_s[:, (e * DX + f) * NPC:(e * DX + f + 1) * NPC],
                        in0=xc_s[:, e * NPC:(e + 1) * NPC],
                        in1=yc_s[:, f * NPC:(f + 1) * NPC],
                    )
            seen_d = set()
            for d, e, f, g in _ENTRIES:
                qsl = q_s[:, (e * DX + f) * NPC:(e * DX + f + 1) * NPC]
                zsl = z_s[:, d * NPC:(d + 1) * NPC]
                if d not in seen_d:
                    seen_d.add(d)
                    nc.vector.tensor_scalar_mul(out=zsl, in0=qsl, scalar1=g)
                else:
                    nc.vector.scalar_tensor_tensor(
                        out=zsl, in0=qsl, scalar=g, in1=zsl,
                        op0=mybir.AluOpType.mult, op1=mybir.AluOpType.add,
                    )

            # ---- stage D: S4 per-l matmuls -> zz [c', (d,n)]
            zz_s = big.tile([128, DZ * NPC], f32, tag="zz_s")
            for l, (b0, b1) in enumerate(zlb):
                w = (b1 - b0) * NPC
                off = b0 * NPC
                for ch0 in range(0, w, 512):
                    cw = min(512, w - ch0)
                    ps = psm.tile([128, 512], f32, tag="psm")
                    nc.tensor.matmul(
                        ps[:, :cw],
                        wz_s[:, l * C:(l + 1) * C],
                        z_s[:, off + ch0: off + ch0 + cw],
                        start=True, stop=True,
                    )
                    nc.vector.tensor_copy(
                        out=zz_s[:, off + ch0: off + ch0 + cw], in_=ps[:, :cw])

            # ---- stage E: transpose per d -> out_s [n, (c,d)]
            out_s = big.tile([128, C * DZ], f32, tag="out_s")
            out_v = out_s[:].rearrange("n (c d) -> n c d", d=DZ)
            for d in range(DZ):
                ps = pst.tile([128, 128], f32, tag="pst")
                nc.tensor.transpose(ps[:], zz_s[:, d * NPC:(d + 1) * NPC], ident)
                nc.vector.tensor_copy(out=out_v[:, :, d], in_=ps[:])

            # ---- stage F: contiguous DMA out ([n, (c,d)] matches DRAM layout)
            nc.sync.dma_start(
                out=out_d[:].rearrange("n c d -> n (c d)"),
                in_=out_s[:],
            )

    # The TPB instruction encoding has two sync-wait slots, but Tile's tail
    # drain aggregates one wait per live proc (PE, DVE, 2 DMA queues = 4).
    # Move the engine waits onto the preceding out-DMA (same SP sequencer,
    # so program order preserves the ordering), leaving <=2 waits each.
    # Walrus accepts a single sync-wait per instruction.  Everything except
    # the out-DMA's queue completion is transitively ordered before the tail
    # drain (PE's first dummy waits the input queue; out-DMA waits DVE-final;
    # the final DVE copy waits PE-final), so the drain keeps only that wait.
    last_dma = None
    for blk in nc.m.functions[0].blocks:
        for ins in blk.instructions:
            si = ins.sync_info
            if type(ins).__name__ == "InstDMACopy":
                last_dma = ins
            elif type(ins).__name__ == "InstDrain" and si and len(si.on_wait) > 1:
                out_q = last_dma.sync_info.on_update[0].ant_name
                keep = [w for w in si.on_wait if w.ant_name == out_q]
                assert len(keep) == 1, (out_q, [w.ant_name for w in si.on_wait])
                si.on_wait = keep

    _NC_CACHE = nc
    return nc


def pack_weights(wx, wy, wz):
    s1 = np.float32(1.0 / math.sqrt(MUL))
    s4 = np.float32(1.0 / math.sqrt(C))
    blocks = (
        [wx[l].astype(np.float32) * s1 for l in range(L1 + 1)]
        + [wy[l].astype(np.float32) * s1 for l in range(L2 + 1)]
        + [wz[l].astype(np.float32) * s4 for l in range(LZ + 1)]
        + [np.eye(128, dtype=np.float32)]
    )
    return np.ascontiguousarray(np.concatenate(blocks, axis=1))


def make_in_maps(x, y, wx, wy, wz):
    w_h = pack_weights(wx, wy, wz)
    in_maps = []
    for i in range(NCORES):
        sl = slice(i * NPC, (i + 1) * NPC)
        blob = np.concatenate([
            np.asarray(x[sl], dtype=np.float32).reshape(NPC, MUL * DX),
            np.asarray(y[sl], dtype=np.float32).reshape(NPC, MUL * DX),
            w_h,
        ], axis=1)
        in_maps.append({"inp": np.ascontiguousarray(blob)})
    return in_maps


def kernel(x, y, wx, wy, wz):
    from concourse.bass_utils import run_bass_kernel_spmd

    nc = _build()
    res = run_bass_kernel_spmd(nc, make_in_maps(x, y, wx, wy, wz),
                               list(range(NCORES)))
    return np.concatenate([r["out"] for r in res.results], axis=0)
